# revision 5
# baseline (speedup 1.0000x reference)
"""Trainium2 Bass kernel for nn_Absolute_attention (sparse_attention).

Reference math (b=4, l=4096, dim=1024, h=16, hd=64):
    q = softmax((x @ Wq.T).reshape(b,l,h,hd+1), -1)
    time encoding: qk_weight = (1-q[...,-1]) * sum_d(time^2)  where
        sum_d(time[l,h,:]^2) = inv_hd * sum_j((c+s)^2 + (c-s)^2) = 2 exactly,
        so qk_weight = 2*(1-q_last)  (time/cos/sin cancel analytically).
    k = softmax((x @ Wk.T).reshape(b,l,h,hd), -1) * mask
    v = x @ Wv.T
    out = ((qk_weight[...,None]*k).reshape(b,l,h*hd) * v) @ Wo.T + bo

Everything is pointwise per (b,l) row -> pure data-parallel row sharding:
16384 rows over 8 cores = 2048 rows/core, 16 blocks of 128 rows.

Q-path precision trick: per head subtract the last softmax slot's weight
row (Wq_hat[j] = Wq[j] - Wq[hd]); then exp(z_last)=1 analytically and
    qk_weight = 2*S/(1+S),  S = sum_j exp(zhat_j)   (1024 cols, not 1040).
The Q logits feed a 65-way near-uniform softmax whose output only enters
via 2*(1-q_last), so fp8 quantization noise (~3% per exp) attenuates to
<0.2% there -> the Q projection runs in fp8 DoubleRow (2 contraction
rows per PE pass = half the passes; measured 2x fp16 on HW). K/V/O stay
fp16 (fp8 there puts ~3-8% noise directly on the output; gate is 2e-2).

Per 128-row block (layout: rows on partitions):
    zq = x8_blk @ Wq_hat.T (fp8 DoubleRow, contraction 1024 as 4x256;
         each 256-col accumulation group owns a full PSUM bank -- PSUM
         accumulation state is per 2KB bank, verified on HW)
    zk|v = x_blk @ [Wk;Wv].T (fp16, contraction in 8 chunks of 128)
    e = exp(zq | zk)  (softmax without max-subtraction -- logits are
        O(+-4), exp is safe in fp32)
    S = rowsum eq per head (16x64); denk = rowsum ek
    G = 2*mask*S / ((1+S)*denk)
    a = e_k * v * G[head-broadcast]   (fp16)
    aT = XBAR DMA transpose of a (8x 128x128 blocks, SBUF->SBUF; off PE)
    out = aT.T @ Wo.T + bo  via PE fp16 matmuls, then DMA out in halves.

The first four blocks run phase-major (Q of blocks 0-3, then K, then V)
so compute overlaps the weight DMA stream in arrival order; late weights
(Wv/Wo/bo) are DMA'd from the *scalar* engine's queue after the first
exp batches, so their transfers can't steal HBM bandwidth from the
urgently-needed Q/K weights (DMA queues progress concurrently, not in
issue order). Later blocks run a software pipeline (DMA-transpose of
block i lands while blocks i+1/i+2 project; final matmul of block i-2
runs between the projections of block i) that keeps the PE stream-bound.
"""
import numpy as np
import ml_dtypes

import concourse.bacc as bacc
import concourse.mybir as mybir
import concourse.tile as tile
from concourse.bass_utils import run_bass_kernel_spmd

FP32 = mybir.dt.float32
F16 = mybir.dt.float16
F8 = mybir.dt.float8e4
AX = mybir.AxisListType.X
ADD = mybir.AluOpType.add
MUL = mybir.AluOpType.mult
EXP = mybir.ActivationFunctionType.Exp
DR = mybir.MatmulPerfMode.DoubleRow

B, L, DIM, H, HD = 4, 4096, 1024, 16, 64
ROWS = B * L                      # 16384
NCORES = 8
CROWS = ROWS // NCORES            # 2048
NBLK = CROWS // 128               # 16
NDC = DIM // 128                  # 8 fp16 contraction chunks
NDQ = DIM // 256                  # 4 fp8 DoubleRow contraction chunks
NQ = H * HD                       # 1024 q-hat cols
NK = H * HD                       # 1024 k cols

WARM = 4                          # blocks processed phase-major at start

_CACHE = {}


def _build():
    nc = bacc.Bacc("TRN2", target_bir_lowering=False, debug=False)
    xt_d = nc.dram_tensor("xt", [NBLK, 128, 1024], F16, kind="ExternalInput").ap()
    x8_d = nc.dram_tensor("x8", [NBLK, 128, 1024], F8, kind="ExternalInput").ap()
    wq_d = nc.dram_tensor("wq8", [128, NDQ, 2, NQ], F8, kind="ExternalInput").ap()
    wk_d = nc.dram_tensor("wtk", [128, NDC * 1024], F16, kind="ExternalInput").ap()
    wv_d = nc.dram_tensor("wtv", [128, NDC * 1024], F16, kind="ExternalInput").ap()
    wo_d = nc.dram_tensor("wo", [128, NDC * 1024], F16, kind="ExternalInput").ap()
    bo_d = nc.dram_tensor("bo", [128, 1024], FP32, kind="ExternalInput").ap()
    m_d = nc.dram_tensor("msk", [128, NBLK], FP32, kind="ExternalInput").ap()
    out_d = nc.dram_tensor("out", [NBLK, 128, 1024], FP32, kind="ExternalOutput").ap()

    with tile.TileContext(nc) as tc:
        with (
            tc.tile_pool(name="const", bufs=1) as cp,
            tc.tile_pool(name="xp", bufs=4) as xp,
            tc.tile_pool(name="x8p", bufs=4) as x8p,
            tc.tile_pool(name="ep", bufs=4) as ep,
            tc.tile_pool(name="t1p", bufs=2) as t1p,
            tc.tile_pool(name="ap_", bufs=3) as apool,
            tc.tile_pool(name="atp", bufs=3) as atp,
            tc.tile_pool(name="op", bufs=2) as op,
            tc.tile_pool(name="sp", bufs=2) as sp,
            tc.tile_pool(name="pp", bufs=6, space="PSUM") as pp,
            tc.tile_pool(name="outp", bufs=2, space="PSUM") as outp,
        ):
            wq8 = cp.tile([128, NDQ, 2, NQ], F8, tag="wq8")
            wtk = cp.tile([128, NDC * 1024], F16, tag="wtk")
            wtv = cp.tile([128, NDC * 1024], F16, tag="wtv")
            wo = cp.tile([128, NDC * 1024], F16, tag="wo")
            bo = cp.tile([128, 1024], FP32, tag="bo")
            msk = cp.tile([128, NBLK], FP32, tag="msk")

            def load_xt(i):
                t = xp.tile([128, 1024], F16, tag="xt")
                nc.sync.dma_start(t[:], xt_d[i])
                return t

            def load_x8(i):
                t = x8p.tile([128, 1024], F8, tag="x8")
                nc.sync.dma_start(t[:], x8_d[i])
                return t

            # Urgent stream on the sync queue: fp8 Q weights + warm x
            # tiles + K weights. Wv/Wo/bo issue later from the scalar
            # engine (see below) so they don't compete for HBM bandwidth.
            x8s = {0: load_x8(0)}
            nc.sync.dma_start(wq8[:], wq_d[:])
            xts = {0: load_xt(0)}
            nc.sync.dma_start(wtk[:, 0:4096], wk_d[:, 0:4096])
            x8s[1] = load_x8(1)
            xts[1] = load_xt(1)
            nc.sync.dma_start(wtk[:, 4096:8192], wk_d[:, 4096:8192])
            x8s[2] = load_x8(2)
            xts[2] = load_xt(2)
            x8s[3] = load_x8(3)
            xts[3] = load_xt(3)
            nc.sync.dma_start(msk[:], m_d[:])

            def proj_q(x8, e):
                """zq-hat in fp8 DoubleRow; exp into e[:, 0:1024].

                PSUM accumulation state is per 2KB bank: two concurrent
                start..stop groups in one bank corrupt each other (verified
                on HW), so each 256-col group gets a full-bank tile."""
                pss = [pp.tile([128, 512], FP32, tag="pp", name="qps")
                       for _ in range(4)]
                for dc in range(NDQ):
                    st = x8[:, dc * 256:(dc + 1) * 256].rearrange(
                        "p (i r) -> p i r", i=2)
                    for t in range(4):
                        nc.tensor.matmul(
                            pss[t][:, 0:256], st,
                            wq8[:, dc, :, t * 256:(t + 1) * 256],
                            start=(dc == 0), stop=(dc == NDQ - 1),
                            perf_mode=DR)
                for t in range(4):
                    nc.scalar.activation(e[:, t * 256:(t + 1) * 256],
                                         pss[t][:, 0:256], EXP)

            def proj_k(xt, e):
                """zk in fp16; exp into e[:, 1024:2048]."""
                for t in range(2):
                    ps = pp.tile([128, 512], FP32, tag="pp", name="kps")
                    for c in range(NDC):
                        lo = (t * NDC + c) * 512
                        nc.tensor.matmul(
                            ps[:], xt[:, c * 128:(c + 1) * 128],
                            wtk[:, lo:lo + 512],
                            start=(c == 0), stop=(c == NDC - 1))
                    nc.scalar.activation(e[:, NQ + t * 512:NQ + (t + 1) * 512],
                                         ps[:], EXP)

            def proj_v(xt, t):
                ps = pp.tile([128, 512], FP32, tag="pp", name="vps")
                for c in range(NDC):
                    lo = (t * NDC + c) * 512
                    nc.tensor.matmul(
                        ps[:], xt[:, c * 128:(c + 1) * 128], wtv[:, lo:lo + 512],
                        start=(c == 0), stop=(c == NDC - 1))
                return ps

            def finish_block(i, xt, e):
                """v matmuls, softmax stats, gate, a = G*ek*v, then kick
                off the XBAR DMA transpose a -> aT (completes ~2 blocks
                before tail_back needs it)."""
                ps5 = proj_v(xt, 0)
                ps6 = proj_v(xt, 1)

                eq = e[:, 0:NQ].rearrange("p (h j) -> p h j", j=HD)
                ek = e[:, NQ:NQ + NK].rearrange("p (h j) -> p h j", j=HD)
                s = sp.tile([128, H], FP32, tag="s")
                denk = sp.tile([128, H], FP32, tag="denk")
                dd = sp.tile([128, H], FP32, tag="dd")
                g = sp.tile([128, H], FP32, tag="g")
                nc.vector.tensor_reduce(s[:], eq, axis=AX, op=ADD)
                nc.vector.tensor_reduce(denk[:], ek, axis=AX, op=ADD)
                nc.vector.tensor_scalar_add(dd[:], s[:], 1.0)      # 1+S
                nc.vector.tensor_mul(dd[:], dd[:], denk[:])        # (1+S)*denk
                nc.vector.reciprocal(dd[:], dd[:])
                # msk holds 2*attention_mask -> G = 2*mask*S/((1+S)*denk)
                nc.vector.scalar_tensor_tensor(
                    g[:], s[:], msk[:, i:i + 1], dd[:], op0=MUL, op1=MUL)

                t1 = t1p.tile([128, 1024], FP32, tag="t1")
                nc.vector.tensor_mul(t1[:, 0:512], e[:, NQ:NQ + 512], ps5[:])
                nc.vector.tensor_mul(t1[:, 512:1024], e[:, NQ + 512:NQ + 1024],
                                     ps6[:])
                a = apool.tile([128, 1024], F16, tag="a")
                nc.vector.tensor_mul(
                    a[:].rearrange("p (h j) -> p h j", j=HD),
                    t1[:].rearrange("p (h j) -> p h j", j=HD),
                    g[:].to_broadcast((128, H, HD)))

                at = atp.tile([128, 1024], F16, tag="at")
                nc.sync.dma_start_transpose(
                    at[:].rearrange("p (c r) -> p c r", c=NDC), a[:])
                return (i, at)

            def tail_back(fr):
                """Final matmul + bias, DMA out in halves."""
                i, at = fr
                outsb = op.tile([128, 1024], FP32, tag="outsb")
                for half in range(2):
                    ops = outp.tile([128, 512], FP32, tag="outp")
                    for c in range(NDC):
                        nc.tensor.matmul(
                            ops[:], at[:, c * 128:(c + 1) * 128],
                            wo[:, c * 1024 + half * 512: c * 1024 + half * 512 + 512],
                            start=(c == 0), stop=(c == NDC - 1))
                    nc.vector.tensor_add(outsb[:, half * 512:(half + 1) * 512],
                                         ops[:], bo[:, half * 512:(half + 1) * 512])
                    nc.sync.dma_start(out_d[i][:, half * 512:(half + 1) * 512],
                                      outsb[:, half * 512:(half + 1) * 512])

            # ---- warmup: blocks 0..WARM-1 phase-major, tracking the
            # weight-stream arrival order (wq8 -> wtk -> wtv -> wo) ----
            es = {i: ep.tile([128, NQ + NK], F16, tag="e", name="e")
                  for i in range(WARM)}
            for i in range(WARM):
                proj_q(x8s[i], es[i])
                if i == 0:
                    # Scalar engine reaches these only after block 0's Q
                    # exps ran -> Wv transfer starts a few us in, after
                    # the urgent wq8/wtk stream has drained.
                    nc.scalar.dma_start(wtv[:, 0:4096], wv_d[:, 0:4096])
                    nc.scalar.dma_start(wtv[:, 4096:8192], wv_d[:, 4096:8192])
                if i == 1:
                    nc.scalar.dma_start(bo[:], bo_d[:])
                    nc.scalar.dma_start(wo[:, 0:4096], wo_d[:, 0:4096])
                    nc.scalar.dma_start(wo[:, 4096:8192], wo_d[:, 4096:8192])
            for i in range(WARM):
                proj_k(xts[i], es[i])
            pending = []
            for i in range(WARM):
                pending.append(finish_block(i, xts[i], es[i]))

            # ---- steady state ----
            for i in range(WARM, NBLK):
                xt = xts.get(i) or load_xt(i)
                x8 = x8s.get(i) or load_x8(i)
                e = ep.tile([128, NQ + NK], F16, tag="e")
                proj_q(x8, e)
                proj_k(xt, e)
                tail_back(pending.pop(0))
                pending.append(finish_block(i, xt, e))
            for fr in pending:
                tail_back(fr)
    nc.compile()
    return nc


def _host_prep(x, attention_mask, Wq, Wk, Wv, Wo, bo):
    x_flat = np.ascontiguousarray(np.asarray(x, dtype=np.float32)).reshape(ROWS, DIM)

    # Wq_hat: per head subtract the last slot's row, drop it -> [1024, 1024]
    Wq_r = np.asarray(Wq, np.float32).reshape(H, HD + 1, DIM)
    Wq_hat = (Wq_r[:, :HD, :] - Wq_r[:, HD:HD + 1, :]).reshape(H * HD, DIM)
    # DoubleRow layout: wq8[p, dc, i, n] = Wq_hat[n, dc*256 + i*128 + p]
    wq8_host = np.ascontiguousarray(
        Wq_hat.T.reshape(NDQ, 2, 128, NQ).transpose(2, 0, 1, 3)
    ).astype(ml_dtypes.float8_e4m3)

    def wcat(WT):
        cols = []
        for t in range(2):
            for c in range(NDC):
                cols.append(WT[c * 128:(c + 1) * 128, t * 512:(t + 1) * 512])
        return np.ascontiguousarray(np.concatenate(cols, axis=1)).astype(np.float16)

    wtk_host = wcat(np.asarray(Wk, np.float32).T)
    wtv_host = wcat(np.asarray(Wv, np.float32).T)

    wo_host = np.ascontiguousarray(
        np.asarray(Wo, np.float32).T.reshape(NDC, 128, 1024)
        .transpose(1, 0, 2).reshape(128, NDC * 1024)).astype(np.float16)
    bo_host = np.ascontiguousarray(
        np.broadcast_to(np.asarray(bo, np.float32), (128, 1024)))
    m_flat = (2.0 * np.asarray(attention_mask, np.float32)).reshape(ROWS)

    in_maps = []
    for i in range(NCORES):
        sl = slice(i * CROWS, (i + 1) * CROWS)
        xt32 = np.ascontiguousarray(
            x_flat[sl].reshape(NBLK, 128, NDC, 128).transpose(0, 3, 2, 1)
        ).reshape(NBLK, 128, 1024)
        xt = xt32.astype(np.float16)
        x8 = xt32.astype(ml_dtypes.float8_e4m3)
        mc = np.ascontiguousarray(m_flat[sl].reshape(NBLK, 128).T)
        in_maps.append({"xt": xt, "x8": x8, "wq8": wq8_host, "wtk": wtk_host,
                        "wtv": wtv_host, "wo": wo_host, "bo": bo_host,
                        "msk": mc})
    return in_maps


def run(inputs, trace=False):
    """Run the kernel; returns (output, exec_time_ns or None)."""
    if "nc" not in _CACHE:
        _CACHE["nc"] = _build()
    nc = _CACHE["nc"]
    in_maps = _host_prep(
        inputs["x"], inputs["attention_mask"], inputs["Wq"], inputs["Wk"],
        inputs["Wv"], inputs["Wo"], inputs["bo"])
    res = None
    for attempt in range(3):
        try:
            res = run_bass_kernel_spmd(nc, in_maps, list(range(NCORES)),
                                       trace=trace)
            break
        except Exception:
            # rare transient NRT_EXEC_UNIT_UNRECOVERABLE; device recovers
            if attempt == 2:
                raise
            import time as _time
            _time.sleep(10)
    out = np.concatenate(
        [res.results[i]["out"].reshape(CROWS, DIM) for i in range(NCORES)],
        axis=0).reshape(B, L, DIM)
    return out, res.exec_time_ns


def kernel(**inputs) -> np.ndarray:
    assert inputs["x"].shape == (B, L, DIM)
    out, _ = run(inputs, trace=False)
    return out


# revision 10
# speedup vs baseline: 1.0322x; 1.0322x over previous
"""Trainium2 Bass kernel for nn_Absolute_attention (sparse_attention).

Reference math (b=4, l=4096, dim=1024, h=16, hd=64):
    q = softmax((x @ Wq.T).reshape(b,l,h,hd+1), -1)
    time encoding: qk_weight = (1-q[...,-1]) * sum_d(time^2)  where
        sum_d(time[l,h,:]^2) = inv_hd * sum_j((c+s)^2 + (c-s)^2) = 2 exactly,
        so qk_weight = 2*(1-q_last)  (time/cos/sin cancel analytically).
    k = softmax((x @ Wk.T).reshape(b,l,h,hd), -1) * mask
    v = x @ Wv.T
    out = ((qk_weight[...,None]*k).reshape(b,l,h*hd) * v) @ Wo.T + bo

Everything is pointwise per (b,l) row -> pure data-parallel row sharding:
16384 rows over 8 cores = 2048 rows/core, 16 blocks of 128 rows.

Q-path precision trick: per head subtract the last softmax slot's weight
row (Wq_hat[j] = Wq[j] - Wq[hd]); then exp(z_last)=1 analytically and
    qk_weight = 2*S/(1+S),  S = sum_j exp(zhat_j)   (1024 cols, not 1040).
The Q logits feed a 65-way near-uniform softmax whose output only enters
via 2*(1-q_last), so fp8 quantization noise (~3% per exp) attenuates to
<0.2% there -> the Q projection runs in fp8 DoubleRow (2 contraction
rows per PE pass = half the passes; measured 2x fp16 on HW). K/V/O stay
fp16 (fp8 there puts ~3-8% noise directly on the output; gate is 2e-2).

Per 128-row block (layout: rows on partitions):
    zq = x8_blk @ Wq_hat.T (fp8 DoubleRow, contraction 1024 as 4x256;
         each 256-col accumulation group owns a full PSUM bank -- PSUM
         accumulation state is per 2KB bank, verified on HW)
    zk|v = x_blk @ [Wk;Wv].T (fp16, contraction in 8 chunks of 128)
    e = exp(zq | zk)  (softmax without max-subtraction -- logits are
        O(+-4), exp is safe in fp32)
    S = rowsum eq per head (16x64); denk = rowsum ek
    G = 2*mask*S / ((1+S)*denk)
    a = e_k * v * G[head-broadcast]   (fp16)
    aT = XBAR DMA transpose of a (8x 128x128 blocks, SBUF->SBUF; off PE)
    out = aT.T @ Wo.T + bo  via PE fp16 matmuls, then DMA out in halves.

The first four blocks run phase-major (Q of blocks 0-3, then K, then V)
so compute overlaps the weight DMA stream in arrival order; late weights
(Wv/Wo/bo) are DMA'd from the *scalar* engine's queue after the first
exp batches, so their transfers can't steal HBM bandwidth from the
urgently-needed Q/K weights (DMA queues progress concurrently, not in
issue order). Later blocks run a software pipeline (DMA-transpose of
block i lands while blocks i+1/i+2 project; final matmul of block i-2
runs between the projections of block i) that keeps the PE stream-bound.
"""
import numpy as np
import ml_dtypes

import concourse.bacc as bacc
import concourse.mybir as mybir
import concourse.tile as tile
from concourse.bass_utils import run_bass_kernel_spmd

FP32 = mybir.dt.float32
F16 = mybir.dt.float16
F8 = mybir.dt.float8e4
AX = mybir.AxisListType.X
ADD = mybir.AluOpType.add
MUL = mybir.AluOpType.mult
EXP = mybir.ActivationFunctionType.Exp
DR = mybir.MatmulPerfMode.DoubleRow

B, L, DIM, H, HD = 4, 4096, 1024, 16, 64
ROWS = B * L                      # 16384
NCORES = 8
CROWS = ROWS // NCORES            # 2048
NBLK = CROWS // 128               # 16
NDC = DIM // 128                  # 8 fp16 contraction chunks
NDQ = DIM // 256                  # 4 fp8 DoubleRow contraction chunks
NQ = H * HD                       # 1024 q-hat cols
NK = H * HD                       # 1024 k cols

WARM = 4                          # blocks processed phase-major at start

_CACHE = {}


def _build():
    nc = bacc.Bacc("TRN2", target_bir_lowering=False, debug=False)
    xt_d = nc.dram_tensor("xt", [NBLK, 128, 1024], F16, kind="ExternalInput").ap()
    x8_d = nc.dram_tensor("x8", [NBLK, 128, 1024], F8, kind="ExternalInput").ap()
    wq_d = nc.dram_tensor("wq8", [128, NDQ, 2, NQ], F8, kind="ExternalInput").ap()
    wk_d = nc.dram_tensor("wtk", [128, NDC * 1024], F16, kind="ExternalInput").ap()
    wv_d = nc.dram_tensor("wtv", [128, NDC * 1024], F16, kind="ExternalInput").ap()
    wo_d = nc.dram_tensor("wo", [128, NDC * 1024], F16, kind="ExternalInput").ap()
    bo_d = nc.dram_tensor("bo", [128, 1024], FP32, kind="ExternalInput").ap()
    m_d = nc.dram_tensor("msk", [128, NBLK], FP32, kind="ExternalInput").ap()
    out_d = nc.dram_tensor("out", [NBLK, 128, 1024], FP32, kind="ExternalOutput").ap()

    with tile.TileContext(nc) as tc:
        with (
            tc.tile_pool(name="const", bufs=1) as cp,
            tc.tile_pool(name="xp", bufs=4) as xp,
            tc.tile_pool(name="x8p", bufs=4) as x8p,
            tc.tile_pool(name="ep", bufs=4) as ep,
            tc.tile_pool(name="t1p", bufs=2) as t1p,
            tc.tile_pool(name="ap_", bufs=3) as apool,
            tc.tile_pool(name="atp", bufs=3) as atp,
            tc.tile_pool(name="op", bufs=2) as op,
            tc.tile_pool(name="sp", bufs=2) as sp,
            tc.tile_pool(name="pp", bufs=6, space="PSUM") as pp,
            tc.tile_pool(name="outp", bufs=2, space="PSUM") as outp,
        ):
            wq8 = cp.tile([128, NDQ, 2, NQ], F8, tag="wq8")
            wtk = cp.tile([128, NDC * 1024], F16, tag="wtk")
            wtv = cp.tile([128, NDC * 1024], F16, tag="wtv")
            wo = cp.tile([128, NDC * 1024], F16, tag="wo")
            bo = cp.tile([128, 1024], FP32, tag="bo")
            msk = cp.tile([128, NBLK], FP32, tag="msk")

            def load_xt(i):
                t = xp.tile([128, 1024], F16, tag="xt")
                nc.sync.dma_start(t[:], xt_d[i])
                return t

            def load_x8(i):
                t = x8p.tile([128, 1024], F8, tag="x8")
                nc.sync.dma_start(t[:], x8_d[i])
                return t

            # DMA queues progress concurrently (not in issue order), so a
            # big transfer issued early steals HBM bandwidth from urgent
            # ones. Only the data block 0 needs right away goes on the
            # sync queue at t0; every later weight is chained on the
            # scalar queue behind a 1-column "anchor" copy that reads the
            # previous transfer's tail -- each link starts only when its
            # predecessor has landed, serializing the stream in need
            # order at full bandwidth.
            x8s = {0: load_x8(0)}
            nc.sync.dma_start(wq8[:], wq_d[:])
            x8s[1] = load_x8(1)
            xts = {0: load_xt(0)}
            dmy = cp.tile([128, 1], F16, tag="dmy")

            def chain(anchor, dmas):
                nc.scalar.copy(dmy[:], anchor)
                for dst, src in dmas:
                    nc.scalar.dma_start(dst, src)

            x8s[2] = x8p.tile([128, 1024], F8, tag="x8", name="x8")
            x8s[3] = x8p.tile([128, 1024], F8, tag="x8", name="x8")
            xts[1] = xp.tile([128, 1024], F16, tag="xt", name="xt")
            xts[2] = xp.tile([128, 1024], F16, tag="xt", name="xt")
            xts[3] = xp.tile([128, 1024], F16, tag="xt", name="xt")
            chain(xts[0][:, 1023:1024],
                  [(wtk[:, 0:4096], wk_d[:, 0:4096]),
                   (x8s[2][:], x8_d[2]), (x8s[3][:], x8_d[3]),
                   (xts[1][:], xt_d[1])])

            def proj_q(x8, e):
                """zq-hat in fp8 DoubleRow; exp into e[:, 0:1024].

                PSUM accumulation state is per 2KB bank: two concurrent
                start..stop groups in one bank corrupt each other (verified
                on HW), so each 256-col group gets a full-bank tile."""
                pss = [pp.tile([128, 512], FP32, tag="pp", name="qps")
                       for _ in range(4)]
                for dc in range(NDQ):
                    st = x8[:, dc * 256:(dc + 1) * 256].rearrange(
                        "p (i r) -> p i r", i=2)
                    for t in range(4):
                        nc.tensor.matmul(
                            pss[t][:, 0:256], st,
                            wq8[:, dc, :, t * 256:(t + 1) * 256],
                            start=(dc == 0), stop=(dc == NDQ - 1),
                            perf_mode=DR)
                for t in range(4):
                    nc.scalar.activation(e[:, t * 256:(t + 1) * 256],
                                         pss[t][:, 0:256], EXP)

            def proj_k_tile(xt, e, t):
                """zk half t in fp16; exp into e[:, 1024+512t : 1024+512(t+1)]."""
                ps = pp.tile([128, 512], FP32, tag="pp", name="kps")
                for c in range(NDC):
                    lo = (t * NDC + c) * 512
                    nc.tensor.matmul(
                        ps[:], xt[:, c * 128:(c + 1) * 128],
                        wtk[:, lo:lo + 512],
                        start=(c == 0), stop=(c == NDC - 1))
                nc.scalar.activation(e[:, NQ + t * 512:NQ + (t + 1) * 512],
                                     ps[:], EXP)

            def proj_k(xt, e):
                proj_k_tile(xt, e, 0)
                proj_k_tile(xt, e, 1)

            def proj_v(xt, t):
                ps = pp.tile([128, 512], FP32, tag="pp", name="vps")
                for c in range(NDC):
                    lo = (t * NDC + c) * 512
                    nc.tensor.matmul(
                        ps[:], xt[:, c * 128:(c + 1) * 128], wtv[:, lo:lo + 512],
                        start=(c == 0), stop=(c == NDC - 1))
                return ps

            def finish_block(i, xt, e, ps5=None):
                """v matmuls, softmax stats, gate, a = G*ek*v, then kick
                off the XBAR DMA transpose a -> aT (completes ~2 blocks
                before tail_back needs it)."""
                if ps5 is None:
                    ps5 = proj_v(xt, 0)
                ps6 = proj_v(xt, 1)

                eq = e[:, 0:NQ].rearrange("p (h j) -> p h j", j=HD)
                ek = e[:, NQ:NQ + NK].rearrange("p (h j) -> p h j", j=HD)
                s = sp.tile([128, H], FP32, tag="s")
                denk = sp.tile([128, H], FP32, tag="denk")
                dd = sp.tile([128, H], FP32, tag="dd")
                g = sp.tile([128, H], FP32, tag="g")
                nc.vector.tensor_reduce(s[:], eq, axis=AX, op=ADD)
                nc.vector.tensor_reduce(denk[:], ek, axis=AX, op=ADD)
                nc.vector.tensor_scalar_add(dd[:], s[:], 1.0)      # 1+S
                nc.vector.tensor_mul(dd[:], dd[:], denk[:])        # (1+S)*denk
                nc.vector.reciprocal(dd[:], dd[:])
                # msk holds 2*attention_mask -> G = 2*mask*S/((1+S)*denk)
                nc.vector.scalar_tensor_tensor(
                    g[:], s[:], msk[:, i:i + 1], dd[:], op0=MUL, op1=MUL)

                t1 = t1p.tile([128, 1024], FP32, tag="t1")
                nc.vector.tensor_mul(t1[:, 0:512], e[:, NQ:NQ + 512], ps5[:])
                nc.vector.tensor_mul(t1[:, 512:1024], e[:, NQ + 512:NQ + 1024],
                                     ps6[:])
                a = apool.tile([128, 1024], F16, tag="a")
                nc.vector.tensor_mul(
                    a[:].rearrange("p (h j) -> p h j", j=HD),
                    t1[:].rearrange("p (h j) -> p h j", j=HD),
                    g[:].to_broadcast((128, H, HD)))

                at = atp.tile([128, 1024], F16, tag="at")
                nc.sync.dma_start_transpose(
                    at[:].rearrange("p (c r) -> p c r", c=NDC), a[:])
                return (i, at)

            def tail_back(fr):
                """Final matmul + bias, DMA out in halves."""
                i, at = fr
                outsb = op.tile([128, 1024], FP32, tag="outsb")
                for half in range(2):
                    ops = outp.tile([128, 512], FP32, tag="outp")
                    for c in range(NDC):
                        nc.tensor.matmul(
                            ops[:], at[:, c * 128:(c + 1) * 128],
                            wo[:, c * 1024 + half * 512: c * 1024 + half * 512 + 512],
                            start=(c == 0), stop=(c == NDC - 1))
                    nc.vector.tensor_add(outsb[:, half * 512:(half + 1) * 512],
                                         ops[:], bo[:, half * 512:(half + 1) * 512])
                    nc.sync.dma_start(out_d[i][:, half * 512:(half + 1) * 512],
                                      outsb[:, half * 512:(half + 1) * 512])

            # ---- warmup: blocks 0..WARM-1 phase-major (all Q, then K
            # half-by-half, then V), tracking the serialized weight
            # stream wq8 -> wtk.h1 -> wtk.h2 -> wtv.h1 -> wtv.h2 -> wo.
            # Chain links are emitted between phases so each anchor's
            # wait is ~zero on the scalar queue. ----
            es = {i: ep.tile([128, NQ + NK], F16, tag="e", name="e")
                  for i in range(WARM)}
            for i in range(WARM):
                proj_q(x8s[i], es[i])
            chain(wtk[:, 4095:4096],
                  [(wtk[:, 4096:8192], wk_d[:, 4096:8192]),
                   (xts[2][:], xt_d[2]), (xts[3][:], xt_d[3])])
            for i in range(WARM):
                proj_k_tile(xts[i], es[i], 0)
            chain(wtk[:, 8191:8192],
                  [(wtv[:, 0:4096], wv_d[:, 0:4096]), (msk[:], m_d[:])])
            for i in range(WARM):
                proj_k_tile(xts[i], es[i], 1)
            chain(wtv[:, 4095:4096],
                  [(wtv[:, 4096:8192], wv_d[:, 4096:8192])])
            chain(wtv[:, 8191:8192],
                  [(bo[:], bo_d[:]), (wo[:, 0:4096], wo_d[:, 0:4096])])
            ps5s = [proj_v(xts[i], 0) for i in range(WARM)]
            chain(wo[:, 4095:4096],
                  [(wo[:, 4096:8192], wo_d[:, 4096:8192])])
            pending = []
            for i in range(WARM):
                pending.append(finish_block(i, xts[i], es[i], ps5s[i]))

            # ---- steady state ----
            for i in range(WARM, NBLK):
                xt = xts.get(i) or load_xt(i)
                x8 = x8s.get(i) or load_x8(i)
                e = ep.tile([128, NQ + NK], F16, tag="e")
                proj_q(x8, e)
                proj_k(xt, e)
                tail_back(pending.pop(0))
                pending.append(finish_block(i, xt, e))
            for fr in pending:
                tail_back(fr)
    nc.compile()
    return nc


def _host_prep(x, attention_mask, Wq, Wk, Wv, Wo, bo):
    x_flat = np.ascontiguousarray(np.asarray(x, dtype=np.float32)).reshape(ROWS, DIM)

    # Wq_hat: per head subtract the last slot's row, drop it -> [1024, 1024]
    Wq_r = np.asarray(Wq, np.float32).reshape(H, HD + 1, DIM)
    Wq_hat = (Wq_r[:, :HD, :] - Wq_r[:, HD:HD + 1, :]).reshape(H * HD, DIM)
    # DoubleRow layout: wq8[p, dc, i, n] = Wq_hat[n, dc*256 + i*128 + p]
    wq8_host = np.ascontiguousarray(
        Wq_hat.T.reshape(NDQ, 2, 128, NQ).transpose(2, 0, 1, 3)
    ).astype(ml_dtypes.float8_e4m3)

    def wcat(WT):
        cols = []
        for t in range(2):
            for c in range(NDC):
                cols.append(WT[c * 128:(c + 1) * 128, t * 512:(t + 1) * 512])
        return np.ascontiguousarray(np.concatenate(cols, axis=1)).astype(np.float16)

    wtk_host = wcat(np.asarray(Wk, np.float32).T)
    wtv_host = wcat(np.asarray(Wv, np.float32).T)

    wo_host = np.ascontiguousarray(
        np.asarray(Wo, np.float32).T.reshape(NDC, 128, 1024)
        .transpose(1, 0, 2).reshape(128, NDC * 1024)).astype(np.float16)
    bo_host = np.ascontiguousarray(
        np.broadcast_to(np.asarray(bo, np.float32), (128, 1024)))
    m_flat = (2.0 * np.asarray(attention_mask, np.float32)).reshape(ROWS)

    in_maps = []
    for i in range(NCORES):
        sl = slice(i * CROWS, (i + 1) * CROWS)
        xt32 = np.ascontiguousarray(
            x_flat[sl].reshape(NBLK, 128, NDC, 128).transpose(0, 3, 2, 1)
        ).reshape(NBLK, 128, 1024)
        xt = xt32.astype(np.float16)
        x8 = xt32.astype(ml_dtypes.float8_e4m3)
        mc = np.ascontiguousarray(m_flat[sl].reshape(NBLK, 128).T)
        in_maps.append({"xt": xt, "x8": x8, "wq8": wq8_host, "wtk": wtk_host,
                        "wtv": wtv_host, "wo": wo_host, "bo": bo_host,
                        "msk": mc})
    return in_maps


def run(inputs, trace=False):
    """Run the kernel; returns (output, exec_time_ns or None)."""
    if "nc" not in _CACHE:
        _CACHE["nc"] = _build()
    nc = _CACHE["nc"]
    in_maps = _host_prep(
        inputs["x"], inputs["attention_mask"], inputs["Wq"], inputs["Wk"],
        inputs["Wv"], inputs["Wo"], inputs["bo"])
    res = None
    for attempt in range(3):
        try:
            res = run_bass_kernel_spmd(nc, in_maps, list(range(NCORES)),
                                       trace=trace)
            break
        except Exception:
            # rare transient NRT_EXEC_UNIT_UNRECOVERABLE; device recovers
            if attempt == 2:
                raise
            import time as _time
            _time.sleep(10)
    out = np.concatenate(
        [res.results[i]["out"].reshape(CROWS, DIM) for i in range(NCORES)],
        axis=0).reshape(B, L, DIM)
    return out, res.exec_time_ns


def kernel(**inputs) -> np.ndarray:
    assert inputs["x"].shape == (B, L, DIM)
    out, _ = run(inputs, trace=False)
    return out
